# revision 47
# baseline (speedup 1.0000x reference)
"""Binarized Conv1d + BatchNorm1d (training mode) on 8 TRN2 NeuronCores.

Reference computation:
    bx  = sign(x)          [B=16, Cin=128, L=8192]
    bw  = sign(weight)     [Cout=128, Cin=128, K=5]
    out = conv1d(bx, bw, stride=1, pad=2) + bias
    out = (out - mean(out, (B,L))) * rsqrt(var(out, (B,L)) + 1e-5)

Sharding: data-parallel over batch, 2 batches per core.  Weights are
replicated.  Per-channel BN statistics are combined with a tiny
AllGather ([128,2] f32 per core: mean and E[x^2] of the local shard).

The conv bias cancels exactly inside training-mode BatchNorm
((conv + b) - mean(conv + b) == conv - mean(conv)), so it is ignored.

Kernel structure per core:
  - dummy AllGather on garbage DRAM as the FIRST gpsimd instruction:
    the first collective pays a ~40 us rendezvous barrier + ~20 us
    CC-path warmup, and the in-order CC stream frees up ~60 us after
    the trigger; triggering with zero dependencies right after the
    Tile preamble moves that whole chain as early as it can go
  - 20 dummy matmuls on an uninitialized SBUF tile right behind the
    preamble: the PE's HAM clock gate needs ~3.4 us of sustained
    activity to lift the 1.2 GHz cold throttle, so the ramp is paid
    on garbage instead of the first real conv tiles
  - weights: DMA split over both HWDGE queues, sign -> bf16,
    PE-transpose each tap to [ci, co] stationary tiles
  - stream x in ramped chunks, sign -> bf16 padded row [128, 8196]
  - conv = 5 accumulated bf16 matmuls per [128, 512] PSUM tile
    (sign values are exact in bf16; products are +-1/0 accumulated in
    f32 PSUM, so the conv result is exact integers)
  - bn_stats on each PSUM tile (DVE), PSUM -> SBUF copy on ACT with
    every third tile's copy on DVE (balances ACT sign+copy work
    against DVE stats work; conv stays PE-bound)
  - bn_aggr -> pack (mean, E[x^2]) -> AllGather(bypass) -> ONE gather
    DMA [128, 8, 2] -> strided reduce -> compact rstd/shift chain
  - normalize in 1024-col chunks over DVE / ACT / GpSimd; each chunk's
    store DMA issues right behind its normalize, split across both
    HWDGE queues so the 8 MiB output drains at the two-queue rate

Rejected experiments (measured): 4-bank PSUM quad drains (DVE/ACT
pacing stalls the PE -> HAM re-throttle, slower); xbar DMA transposes
for the weights (6-8 us each, block the sync queue); remote_dma
peer-to-peer stats exchange instead of the collective (the runtime
rejects it -- would otherwise remove the entire ~60 us CC warmup
chain); sign(x) on GpSimd via min/max (far too slow, stalls PE).
"""

import os
import sys

import numpy as np

# concourse is normally importable from the axon site; fall back to the
# staged repo copies if not
try:
    import concourse  # noqa: F401
except ImportError:
    for _p in ("/opt/trn_rl_repo", "/root/.axon_site/_ro/trn_rl_repo"):
        if os.path.isdir(_p):
            sys.path.insert(0, _p)
            break

B = 16
B_LOC = 2
CI = 128
CO = 128
L = 8192
K = 5
PAD = 2
EPS = 1e-5
N_CORES = 8
FREE = 512          # PSUM tile free dim (one bank of f32)
NT = L // FREE      # 16 conv tiles per batch row
XCH = 1024          # out DMA chunk columns (512 KiB per transfer)

_CACHE = {}


def _build_nc():
    import concourse.bacc as bacc
    import concourse.bass as bass
    import concourse.tile as tile
    from concourse import mybir
    from concourse.masks import make_identity

    f32 = mybir.dt.float32
    bf16 = mybir.dt.bfloat16
    Sign = mybir.ActivationFunctionType.Sign
    Sqrt = mybir.ActivationFunctionType.Sqrt
    Copy = mybir.ActivationFunctionType.Copy

    nc = bacc.Bacc("TRN2", target_bir_lowering=False, debug=False, num_devices=N_CORES)

    x = nc.declare_dram_parameter("x", [B_LOC, CI, L], f32, isOutput=False)
    w = nc.declare_dram_parameter("weight", [CO, CI, K], f32, isOutput=False)
    out = nc.declare_dram_parameter("out", [B_LOC, CO, L], f32, isOutput=True)

    with tile.TileContext(nc) as tc:
        with (
            tc.tile_pool(name="singles", bufs=1) as singles,
            tc.tile_pool(name="xin", bufs=1) as xin,
            tc.tile_pool(name="bxp", bufs=2) as bxp_pool,
            tc.tile_pool(name="psum", bufs=8, space="PSUM") as psum,
            tc.tile_pool(name="dram", bufs=2, space="DRAM") as dram,
        ):
            # ---- warm-up collective: the very first gpsimd instruction ----
            # Contents are irrelevant (bypass op, output unused), so no
            # memset / staging DMA: the trigger has zero dependencies and
            # fires the moment the Tile preamble ends.
            warm_in = dram.tile([1, 8], f32)
            warm_out = dram.tile([N_CORES, 8], f32)
            nc.gpsimd.collective_compute(
                "AllGather",
                mybir.AluOpType.bypass,
                replica_groups=[list(range(N_CORES))],
                ins=[warm_in[:].opt()],
                outs=[warm_out[:].opt()],
            )

            # ---- weight + first x chunk DMAs issued before anything else ----
            # weight halves on both HWDGE queues so sign(w) starts ~2 us
            # earlier than a single 330 KiB transfer would allow
            wf32 = singles.tile([CO, CI, K], f32)
            nc.sync.dma_start(out=wf32[:, 0:64, :], in_=w[:, 0:64, :])
            nc.scalar.dma_start(out=wf32[:, 64:128, :], in_=w[:, 64:128, :])
            xts = []
            for b in range(B_LOC):
                xts.append(
                    xin.tile([CI, L], f32, tag=f"xt{b}", name=f"xt{b}")
                )
            nc.sync.dma_start(out=xts[0][:, 0:512], in_=x[0, :, 0:512])

            # ---- weights: sign -> bf16, transpose each tap to [ci, co] ----
            ident = singles.tile([128, 128], bf16)
            make_identity(nc, ident)

            wsgn = singles.tile([CO, CI, K], bf16)
            nc.scalar.activation(out=wsgn, in_=wf32, func=Sign)

            # dummy matmuls on an uninitialized SBUF tile (values are
            # irrelevant, the PSUM bank is overwritten later): zero
            # dependencies, so they start the moment the Tile preamble
            # ends.  The PE's HAM clock gate needs ~3.4 us of sustained
            # activity to lift the 1.2 GHz cold throttle -- warm it up
            # before the first real matmul instead of paying the
            # half-clock ramp on real work.
            warm_mm = singles.tile([128, 128], bf16)
            nc.vector.memset(warm_mm, 0.0)
            warm_ps = psum.tile([128, FREE], f32, tag="pt")
            for _ in range(26):
                nc.tensor.matmul(
                    warm_ps[:, 0:128], lhsT=warm_mm, rhs=warm_mm,
                    start=True, stop=True,
                )

            wT = singles.tile([CI, K, CO], bf16)  # stationary tiles per tap
            for k in range(K):
                pw = psum.tile([CI, CO], bf16, tag="pt")
                nc.tensor.transpose(pw, wsgn[:, :, k], ident)
                nc.vector.tensor_copy(out=wT[:, k, :], in_=pw)

            # ---- conv + local stats ----
            # conv output kept resident in SBUF: [128 co, B_LOC * L] f32
            conv_sb = singles.tile([CO, B_LOC, L], f32)
            stats = singles.tile([CO, B_LOC * NT, 6], f32)

            # ramped DMA chunks: small first chunk (already issued above
            # for b=0) so the first matmuls start early; sign emitted per
            # <=1024 cols so matmuls chase the conversion closely.
            # b0/b1 chunks are INTERLEAVED on the sync queue: with b0's
            # 4 MiB strictly first, b1's data only lands ~21 us in, b1's
            # sign starts late, and the PE runs dry for ~6 us at the
            # b0->b1 seam -- the HAM clock gate re-throttles to 1.2 GHz
            # (measured: K=4/8 from 29.8-36.6 us).  Interleaving keeps the
            # PE fed continuously.
            CHUNK_SCHED = [
                [512, 512, 1024, 2048, 2048, 2048],
                [2048, 2048, 2048, 1024, 512, 512],
            ]
            # sync queue: all of b0 (in order, feeding the PE start), then
            # b1's first chunks; b1's tail chunks ride the scalar queue and
            # land early, so b1's sign never gates the PE
            ISSUE_SYNC = [(0, 1), (0, 2), (0, 3), (0, 4), (0, 5),
                          (1, 0), (1, 1), (1, 2)]
            ISSUE_SCALAR = [(1, 3), (1, 4), (1, 5)]
            offs = [
                [sum(CHUNK_SCHED[b][:i]) for i in range(len(CHUNK_SCHED[b]))]
                for b in range(B_LOC)
            ]
            for b, ci_ in ISSUE_SYNC:
                off, ch = offs[b][ci_], CHUNK_SCHED[b][ci_]
                nc.sync.dma_start(
                    out=xts[b][:, off : off + ch], in_=x[b, :, off : off + ch]
                )
            for b, ci_ in ISSUE_SCALAR:
                off, ch = offs[b][ci_], CHUNK_SCHED[b][ci_]
                nc.scalar.dma_start(
                    out=xts[b][:, off : off + ch], in_=x[b, :, off : off + ch]
                )
            for b in range(B_LOC):
                bxp = bxp_pool.tile([CI, L + 2 * PAD], bf16)
                nc.vector.memset(bxp[:, 0:PAD], 0.0)
                nc.vector.memset(bxp[:, L + PAD : L + 2 * PAD], 0.0)
                # one staging tile per batch, written once in disjoint
                # chunks -> no DMA ever needs a buffer-reuse wait (HW-queue
                # DMAs only support a single sync wait)
                xt = xts[b]
                off = 0
                for ci_, ch in enumerate(CHUNK_SCHED[b]):
                    s = off
                    while s < off + ch:
                        sw = min(1024, off + ch - s)
                        nc.scalar.activation(
                            out=bxp[:, PAD + s : PAD + s + sw],
                            in_=xt[:, s : s + sw],
                            func=Sign,
                        )
                        s += sw
                    off += ch
                for t in range(NT):
                    pt = psum.tile([CO, FREE], f32, tag="pt")
                    for k in range(K):
                        nc.tensor.matmul(
                            pt,
                            lhsT=wT[:, k, :],
                            rhs=bxp[:, t * FREE + k : t * FREE + k + FREE],
                            start=(k == 0),
                            stop=(k == K - 1),
                        )
                    nc.vector.bn_stats(out=stats[:, b * NT + t, :], in_=pt)
                    dst = conv_sb[:, b, t * FREE : (t + 1) * FREE]
                    # 14 of 32 copies on DVE: measured per-copy cost is the
                    # same on both engines (~0.7 us/512), so this equalizes
                    # ACT (sign + 18 copies) against DVE (stats + 14)
                    if (b * NT + t) % 16 in (1, 3, 5, 8, 10, 12, 14):
                        nc.vector.tensor_copy(out=dst, in_=pt)
                    else:
                        nc.scalar.activation(out=dst, in_=pt, func=Copy)

            # ---- global stats: all-reduce (mean, E[x^2]) sums ----
            # bn_aggr writes (mean, var); turn the var slot into E[x^2] in
            # place; the /N_CORES is folded into the post-AR chain
            pk = singles.tile([CO, 2], f32)
            sq = singles.tile([CO, 1], f32)
            nc.vector.bn_aggr(out=pk, in_=stats)
            nc.vector.tensor_mul(sq, pk[:, 0:1], pk[:, 0:1])
            nc.vector.tensor_add(pk[:, 1:2], pk[:, 1:2], sq)

            # AllGather ([128,2] per core -> [8*128,2]) has a lower floor
            # than AllReduce; the 8-way sum is done locally on DVE
            cc_in = dram.tile([CO, 2], f32)
            cc_out = dram.tile([N_CORES * CO, 2], f32)
            nc.sync.dma_start(out=cc_in, in_=pk)
            nc.gpsimd.collective_compute(
                "AllGather",
                mybir.AluOpType.bypass,
                replica_groups=[list(range(N_CORES))],
                ins=[cc_in[:].opt()],
                outs=[cc_out[:].opt()],
            )
            # one gather DMA: [8*CO, 2] dram -> [CO, 8, 2] sbuf
            gsum = singles.tile([CO, N_CORES, 2], f32)
            nc.sync.dma_start(
                out=gsum, in_=cc_out.rearrange("(r p) c -> p r c", p=CO)
            )
            gst = singles.tile([CO, 2], f32)
            nc.vector.reduce_sum(
                out=gst,
                in_=gsum.rearrange("p r c -> p c r"),
                axis=mybir.AxisListType.X,
            )

            # gvar = E2sum/8 - (sum/8)^2 ; rstd = 1/sqrt(gvar + eps)
            # shift = -(sum/8)*rstd = (sum * rstd) * (-1/8)
            gm2 = singles.tile([CO, 1], f32)
            gvar = singles.tile([CO, 1], f32)
            sd = singles.tile([CO, 1], f32)
            rstd = singles.tile([CO, 1], f32)
            shift = singles.tile([CO, 1], f32)
            eps_t = singles.tile([CO, 1], f32)
            nc.vector.memset(eps_t, EPS)
            nc.vector.tensor_scalar(
                out=gm2,
                in0=gst[:, 0:1],
                scalar1=gst[:, 0:1],
                scalar2=1.0 / (N_CORES * N_CORES),
                op0=mybir.AluOpType.mult,
                op1=mybir.AluOpType.mult,
            )
            nc.vector.tensor_scalar(
                out=gvar,
                in0=gst[:, 1:2],
                scalar1=1.0 / N_CORES,
                scalar2=gm2[:, 0:1],
                op0=mybir.AluOpType.mult,
                op1=mybir.AluOpType.subtract,
            )
            nc.scalar.activation(out=sd, in_=gvar, func=Sqrt, bias=eps_t[:, 0:1])
            nc.vector.reciprocal(rstd, sd)
            nc.vector.tensor_scalar(
                out=shift,
                in0=gst[:, 0:1],
                scalar1=rstd[:, 0:1],
                scalar2=-1.0 / N_CORES,
                op0=mybir.AluOpType.mult,
                op1=mybir.AluOpType.mult,
            )

            # ---- normalize (in place) + store ----
            # distribute the x*rstd+shift pass across DVE / ACT / GpSimd so
            # the store phase is DMA-bound instead of DVE-paced; each
            # chunk's store DMA issues right behind its normalize (ACT
            # chunks store on the scalar HWDGE queue, the rest on sync)
            Ident = mybir.ActivationFunctionType.Identity
            ENG_SCHED = [0, 1, 2, 0, 1, 0, 0, 1, 2, 0, 1, 0, 0, 1, 2, 0]
            idx = 0
            for b in range(B_LOC):
                for c in range(L // XCH):
                    sl = conv_sb[:, b, c * XCH : (c + 1) * XCH]
                    eng = ENG_SCHED[idx % len(ENG_SCHED)]
                    if eng == 0:
                        nc.vector.tensor_scalar(
                            out=sl,
                            in0=sl,
                            scalar1=rstd[:, 0:1],
                            scalar2=shift[:, 0:1],
                            op0=mybir.AluOpType.mult,
                            op1=mybir.AluOpType.add,
                        )
                    elif eng == 1:
                        nc.scalar.activation(
                            out=sl,
                            in_=sl,
                            func=Ident,
                            bias=shift[:, 0:1],
                            scale=rstd[:, 0:1],
                        )
                    else:
                        nc.gpsimd.tensor_scalar(
                            out=sl,
                            in0=sl,
                            scalar1=rstd[:, 0:1],
                            scalar2=shift[:, 0:1],
                            op0=mybir.AluOpType.mult,
                            op1=mybir.AluOpType.add,
                        )
                    # stores split across both HWDGE queues (DVE chunks ->
                    # sync, ACT/GpSimd chunks -> scalar) so the 8 MiB
                    # output drains at the two-queue rate
                    deng = nc.sync if eng == 0 else nc.scalar
                    deng.dma_start(
                        out=out[b, :, c * XCH : (c + 1) * XCH], in_=sl
                    )
                    idx += 1

    nc.compile()
    return nc


def _run(inputs, trace=False):
    from concourse import bass_utils

    x = np.ascontiguousarray(np.asarray(inputs["x"], dtype=np.float32))
    weight = np.ascontiguousarray(np.asarray(inputs["weight"], dtype=np.float32))

    if "nc" not in _CACHE:
        _CACHE["nc"] = _build_nc()
    nc = _CACHE["nc"]

    in_maps = [
        {"x": x[i * B_LOC : (i + 1) * B_LOC], "weight": weight}
        for i in range(N_CORES)
    ]
    res = bass_utils.run_bass_kernel_spmd(
        nc, in_maps, core_ids=list(range(N_CORES)), trace=trace
    )
    out = np.concatenate(
        [res.results[i]["out"] for i in range(N_CORES)], axis=0
    ).astype(np.float32)
    return out, res


def kernel(**inputs) -> np.ndarray:
    out, _ = _run(inputs, trace=False)
    return out
